# revision 1
# baseline (speedup 1.0000x reference)
"""Learnable 3D Gaussian field evaluation on 8 Trainium2 NeuronCores.

Reference computes, for B=32768 points x and N=4096 gaussians
(mean m_n, packed Cholesky cov_tril, weight w_n):

    out[b] = sum_n w_n * exp(-0.5 * (x_b - m_n)^T A_n (x_b - m_n)),
    A_n = (L_n L_n^T)^{-1}

Key reformulation: the exponent is a quadratic form in x, so with a
10-dim feature vector f(x) = [x0^2, x1^2, x2^2, x0x1, x0x2, x1x2,
x0, x1, x2, 1] (x centered) and per-gaussian coefficients c_n (with
-0.5 and log w_n folded in),

    exponent[b, n] = f(x_b) . c_n

i.e. a [B,10] x [10,N] matmul on the TensorEngine, followed by
exp + row-sum (fused in one ScalarEngine activation with accum_out).

Precision: PE bf16 matmul at 1 cycle/row; fp32 operands are split into
3 bf16 components each and the 6 significant cross products stacked
along the contraction dim (K=60, free on the PE) -> ~fp32 precision.

Sharding: B axis data-parallel across 8 cores (4096 points each),
gaussian coefficients replicated. No collectives.
"""

import sys

import numpy as np

try:
    import concourse.bass as bass  # noqa: F401
except ImportError:
    sys.path.insert(0, "/opt/trn_rl_repo")

import ml_dtypes

import concourse.bacc as bacc
import concourse.bass as bass  # noqa: F401
import concourse.mybir as mybir
import concourse.tile as tile
from concourse.bass_utils import run_bass_kernel_spmd

B, N = 32768, 4096
N_CORES = 8
B_SHARD = B // N_CORES          # 4096 points per core
PT_TILES = B_SHARD // 128       # 32 point-tiles of 128 points
N_HALF = N // 2                 # ACT chunk: 2048 columns (4 PSUM banks)
KSPLIT = 60                     # 6 bf16 cross products x 10 features
CENTER = 5.0

BF16 = mybir.dt.bfloat16
F32 = mybir.dt.float32


# ---------------------------------------------------------------- host math

def _build_coeffs(means, cov_tril, weights):
    """[N, 10] float64 coefficients c_n so that exponent = f(x') . c_n."""
    m = means.astype(np.float64) - CENTER
    ct = cov_tril.astype(np.float64)
    w = weights.astype(np.float64)
    eps = 1e-6
    L00 = np.exp(ct[:, 0]) + eps
    L11 = np.exp(ct[:, 2]) + eps
    L22 = np.exp(ct[:, 5]) + eps
    L10, L20, L21 = ct[:, 1], ct[:, 3], ct[:, 4]
    i00 = 1.0 / L00
    i11 = 1.0 / L11
    i22 = 1.0 / L22
    i10 = -L10 / (L00 * L11)
    i21 = -L21 / (L11 * L22)
    i20 = (L10 * L21 - L20 * L11) / (L00 * L11 * L22)
    A00 = i00 * i00 + i10 * i10 + i20 * i20
    A01 = i10 * i11 + i20 * i21
    A02 = i20 * i22
    A11 = i11 * i11 + i21 * i21
    A12 = i21 * i22
    A22 = i22 * i22
    Am0 = A00 * m[:, 0] + A01 * m[:, 1] + A02 * m[:, 2]
    Am1 = A01 * m[:, 0] + A11 * m[:, 1] + A12 * m[:, 2]
    Am2 = A02 * m[:, 0] + A12 * m[:, 1] + A22 * m[:, 2]
    mAm = m[:, 0] * Am0 + m[:, 1] * Am1 + m[:, 2] * Am2
    return np.stack(
        [
            -0.5 * A00, -0.5 * A11, -0.5 * A22,
            -A01, -A02, -A12,
            Am0, Am1, Am2,
            -0.5 * mAm + np.log(w),
        ],
        axis=1,
    )


def _build_feats(x):
    """[B, 10] float64 features of centered x."""
    xc = x.astype(np.float64) - CENTER
    x0, x1, x2 = xc[:, 0], xc[:, 1], xc[:, 2]
    return np.stack(
        [x0 * x0, x1 * x1, x2 * x2, x0 * x1, x0 * x2, x1 * x2,
         x0, x1, x2, np.ones_like(x0)],
        axis=1,
    )


def _split3_bf16(a64):
    """float64 -> three bf16 components with p0+p1+p2 covering ~24 bits."""
    p0 = a64.astype(ml_dtypes.bfloat16)
    r1 = a64 - p0.astype(np.float64)
    p1 = r1.astype(ml_dtypes.bfloat16)
    r2 = r1 - p1.astype(np.float64)
    p2 = r2.astype(ml_dtypes.bfloat16)
    return p0, p1, p2


def _device_operands(inputs):
    """Build paired lhsT [120, B/2] plus hi/lo zero-padded rhs [120, N].

    The 60-row split-feature blocks of two adjacent point-tiles are stacked
    into one 120-row stationary operand (the PE array has 128 rows), so a
    single weight load serves both tiles' matmuls. Which tile a matmul
    computes is selected by the rhs: coeffs_hi carries the coefficients in
    rows 0-59 (zeros below), coeffs_lo in rows 60-119 (zeros above)."""
    f = _build_feats(inputs["x"])                                    # [B,10]
    c = _build_coeffs(inputs["means"], inputs["cov_tril"], inputs["weights"])
    F0, F1, F2 = _split3_bf16(f)
    C0, C1, C2 = _split3_bf16(c)
    # products with combined precision loss <= 2^-24: (Fi, Cj), i+j <= 2
    pairs = [(F0, C0), (F0, C1), (F1, C0), (F0, C2), (F1, C1), (F2, C0)]
    lhsT = np.ascontiguousarray(
        np.concatenate([p[0].T for p in pairs], axis=0)
    ).astype(ml_dtypes.bfloat16)                                     # [60, B]
    rhs = np.ascontiguousarray(
        np.concatenate([p[1].T for p in pairs], axis=0)
    ).astype(ml_dtypes.bfloat16)                                     # [60, N]
    # stack adjacent point-tiles 64-partition-aligned: tile A in rows 0-59,
    # tile B in rows 64-123, zero padding rows 60-63 / 124-127
    t = lhsT.reshape(KSPLIT, B // 128, 128)
    lhsT_pair = np.zeros((128, (B // 256) * 128), dtype=ml_dtypes.bfloat16)
    lhsT_pair[:KSPLIT] = t[:, 0::2, :].reshape(KSPLIT, -1)
    lhsT_pair[64:64 + KSPLIT] = t[:, 1::2, :].reshape(KSPLIT, -1)
    return lhsT_pair, rhs


# ------------------------------------------------------------- device kernel

def _dedup_ldweights(nc):
    """Remove redundant InstLdweights: consecutive matmuls reusing the same
    stationary operand only need the first load. Only drops loads that carry
    no semaphore waits/updates and whose weights AP matches the previous
    load, with nothing but matmuls in between on the PE stream."""
    removed = 0
    for blk in nc.m.functions[0].blocks:
        keep = []
        last_sig = None
        for ins in blk.instructions:
            if getattr(ins, "engine", None) == mybir.EngineType.PE:
                tname = type(ins).__name__
                if tname == "InstLdweights":
                    sig = repr(ins.ins[0])
                    if sig == last_sig and ins.sync_info is None:
                        removed += 1
                        continue
                    last_sig = sig
                elif tname != "InstMatmult":
                    last_sig = None
            keep.append(ins)
        if removed:
            del blk.instructions[:]
            for ins in keep:
                blk.instructions.append(ins)
    return removed


_ENGINE_SEM_PREFIX = {
    mybir.EngineType.PE: "PE_",
    mybir.EngineType.Activation: "Activation_",
}


def _strip_self_waits(nc):
    """Drop same-engine semaphore waits from multi-wait PE/ACT instructions.

    Engines execute their instruction streams in order, so a wait on the
    engine's own completion semaphore is redundant whenever the instruction
    also carries the cross-engine wait that actually orders it. Removing
    them keeps every instruction at <=1 wait, so the compiler does not have
    to materialize extra event-semaphore instructions."""
    n = 0
    for blk in nc.m.functions[0].blocks:
        for ins in blk.instructions:
            pfx = _ENGINE_SEM_PREFIX.get(getattr(ins, "engine", None))
            si = ins.sync_info
            if pfx is None or si is None or not si.on_wait:
                continue
            waits = list(si.on_wait)
            if len(waits) < 2:
                continue
            kept = [w for w in waits if not w.ant_name.startswith(pfx)]
            if kept and len(kept) < len(waits):
                si.on_wait = kept
                n += len(waits) - len(kept)
    return n


def _thin_mm_sem_updates(nc):
    """Coalesce per-matmul semaphore increments: within each run of
    consecutive matmuls on the PE stream, move all PE-semaphore increments
    onto the run's last matmul (summed update_value). Valid because PE
    executes in order and every downstream wait threshold is a multiple of
    the full run's increment total (verified below)."""
    # collect all waits on PE semaphores to verify thresholds stay reachable
    run_lens = set()
    for blk in nc.m.functions[0].blocks:
        run = []
        for ins in blk.instructions:
            if getattr(ins, "engine", None) == mybir.EngineType.PE \
                    and type(ins).__name__ == "InstMatmult":
                run.append(ins)
                continue
            if len(run) > 1:
                run_lens.add(len(run))
            run = []
        if len(run) > 1:
            run_lens.add(len(run))
    if len(run_lens) != 1:
        return 0
    run_len = run_lens.pop()
    for blk in nc.m.functions[0].blocks:
        for ins in blk.instructions:
            si = ins.sync_info
            if si is None or not si.on_wait:
                continue
            for w in si.on_wait:
                if w.ant_name.startswith("PE_") and w.wait_value % run_len:
                    return 0  # mid-run threshold exists; unsafe
    moved = 0
    for blk in nc.m.functions[0].blocks:
        run = []

        def flush(run):
            nonlocal moved
            if len(run) < 2:
                return
            total = 0
            carrier = None
            for mm in run:
                si = mm.sync_info
                if si is None or not si.on_update:
                    continue
                upds = [u for u in si.on_update
                        if u.ant_name.startswith("PE_")]
                if len(upds) != 1 or len(si.on_update) != 1:
                    return  # unexpected shape; skip this run
                total += upds[0].update_value
                carrier = upds[0]
            if total == 0 or carrier is None:
                return
            last_si = run[-1].sync_info
            if last_si is None or not last_si.on_update:
                return
            for mm in run[:-1]:
                si = mm.sync_info
                if si is not None and si.on_update:
                    si.on_update = []
                    moved += 1
            last_si.on_update[0].update_value = total

        for ins in blk.instructions:
            if getattr(ins, "engine", None) == mybir.EngineType.PE \
                    and type(ins).__name__ == "InstMatmult":
                run.append(ins)
                continue
            flush(run)
            run = []
        flush(run)
    return moved


def _strip_dead_const_memsets(nc):
    """Delete framework const-AP memsets whose tensor is never read
    (walrus flags them as no-reader memory locations)."""
    read = set()
    for blk in nc.m.functions[0].blocks:
        for ins in blk.instructions:
            for arg in getattr(ins, "ins", []) or []:
                ref = getattr(arg, "memref", None)
                if ref:
                    read.add(ref)
    removed = 0
    for blk in nc.m.functions[0].blocks:
        keep = []
        for ins in blk.instructions:
            if (type(ins).__name__ == "InstMemset"
                    and ins.sync_info is None
                    and getattr(ins.outs[0], "memref", "").startswith("const-")
                    and ins.outs[0].memref not in read):
                removed += 1
                continue
            keep.append(ins)
        if removed:
            del blk.instructions[:]
            for ins in keep:
                blk.instructions.append(ins)
    return removed


def _trim_tail_barrier(nc):
    """Drop the second all-engine barrier round at the kernel tail.

    The TileContext epilogue runs barrier -> semaphore reset -> barrier.
    The second barrier only fences engines against code that would run
    after the reset; this kernel's end block is the last block, so there
    is nothing to fence. Only plain Drain/EventSemaphore instructions
    after the Pool RANGE_CLEAR are removed; anything unexpected aborts."""
    for blk in nc.m.functions[0].blocks:
        if not getattr(blk, "name", "").endswith("_end"):
            continue
        insts = list(blk.instructions)
        idx = None
        for i, ins in enumerate(insts):
            if (type(ins).__name__ == "InstISA"
                    and ins.engine == mybir.EngineType.Pool):
                idx = i
        if idx is None or idx + 1 >= len(insts):
            return 0
        tail = insts[idx + 1:]
        if any(type(t).__name__ not in ("InstDrain", "InstEventSemaphore")
               for t in tail):
            return 0
        del blk.instructions[:]
        for ins in insts[:idx + 1]:
            blk.instructions.append(ins)
        return len(tail)
    return 0


def _build_bass(repeats=1):
    nc = bacc.Bacc("TRN2", target_bir_lowering=False, debug=False,
                   num_devices=N_CORES)
    feats = nc.dram_tensor("feats", [128, B_SHARD // 2], BF16,
                           kind="ExternalInput")
    coeffs = nc.dram_tensor("coeffs", [KSPLIT, N], BF16,
                            kind="ExternalInput")
    out = nc.dram_tensor("out", [128, PT_TILES], F32,
                         kind="ExternalOutput")

    with tile.TileContext(nc) as tc:
        with (
            tc.tile_pool(name="const", bufs=1) as const_pool,
            tc.tile_pool(name="psum", bufs=1, space="PSUM") as psum_pool,
            tc.tile_pool(name="scratch", bufs=1) as scratch_pool,
            tc.tile_pool(name="acc", bufs=1) as acc_pool,
        ):
            F = const_pool.tile([128, B_SHARD // 2], BF16, tag="F")
            nc.sync.dma_start(F[:], feats.ap())
            # zero-padded hi/lo coefficient operands assembled on device:
            # rows 0-59 select a pair's first tile, rows 64-123 its second
            # (zero rows contribute exactly nothing to the contraction)
            Chi = const_pool.tile([128, N], BF16, tag="Chi")
            nc.vector.memset(Chi[:], 0.0)
            nc.sync.dma_start(Chi[:KSPLIT, :], coeffs.ap())
            Clo = const_pool.tile([128, N], BF16, tag="Clo")
            nc.vector.memset(Clo[:], 0.0)
            nc.sync.dma_start(Clo[64:64 + KSPLIT, :], coeffs.ap())
            accs = acc_pool.tile([128, PT_TILES], F32, tag="accs")

            for _r in range(repeats):
                for p in range(PT_TILES // 2):
                    lhsT = F[:, p * 128:(p + 1) * 128]
                    for half, C in ((0, Chi), (1, Clo)):
                        ps = psum_pool.tile([128, N], F32, tag="ps")
                        for j in range(N // 512):
                            nc.tensor.matmul(
                                ps[:, j * 512:(j + 1) * 512],
                                lhsT,
                                C[:, j * 512:(j + 1) * 512],
                                start=True,
                                stop=True,
                            )
                        sc = scratch_pool.tile([128, N], F32, tag="sc")
                        nc.scalar.activation(
                            sc[:], ps[:], mybir.ActivationFunctionType.Exp,
                            accum_out=accs[:, 2 * p + half:2 * p + half + 1],
                        )
            nc.sync.dma_start(out.ap(), accs[:])
    _dedup_ldweights(nc)
    _strip_self_waits(nc)
    # Note: coalescing per-matmul sem increments onto the last matmul of each
    # run (_thin_mm_sem_updates) is logically sound but trips CoreSim's
    # Tile-metadata replay, and embedded updates measure ~free — not used.
    nc.compile()
    _trim_tail_barrier(nc)
    _strip_dead_const_memsets(nc)
    return nc


# ----------------------------------------------------------------- interface

def _in_maps(inputs):
    lhsT_pair, rhs = _device_operands(inputs)
    half_shard = B_SHARD // 2
    return [
        {
            "feats": np.ascontiguousarray(
                lhsT_pair[:, c * half_shard:(c + 1) * half_shard]
            ),
            "coeffs": rhs,
        }
        for c in range(N_CORES)
    ]


def _run(inputs, trace=False):
    in_maps = _in_maps(inputs)
    nc = _build_bass()
    res = run_bass_kernel_spmd(
        nc, in_maps, core_ids=list(range(N_CORES)), trace=trace
    )
    out_full = np.empty(B, dtype=np.float32)
    for c in range(N_CORES):
        accs = res.results[c]["out"]                   # [128, PT_TILES]
        out_full[c * B_SHARD:(c + 1) * B_SHARD] = accs.T.ravel()
    return out_full, res


def kernel(x, means, cov_tril, weights):
    x = np.asarray(x)
    means = np.asarray(means)
    cov_tril = np.asarray(cov_tril)
    weights = np.asarray(weights)
    assert x.shape == (B, 3) and means.shape == (N, 3)
    assert cov_tril.shape == (N, 6) and weights.shape == (N,)
    out, _ = _run(
        {"x": x, "means": means, "cov_tril": cov_tril, "weights": weights}
    )
    return out



# revision 5
# speedup vs baseline: 580.2025x; 580.2025x over previous
"""Learnable 3D Gaussian field evaluation on 8 Trainium2 NeuronCores.

Reference computes, for B=32768 points x and N=4096 gaussians
(mean m_n, packed Cholesky cov_tril, weight w_n):

    out[b] = sum_n w_n * exp(-0.5 * (x_b - m_n)^T A_n (x_b - m_n)),
    A_n = (L_n L_n^T)^{-1}

Two key reformulations:

1. Quadratic-form matmul: the exponent is a quadratic in x, so with a
   10-dim feature vector f(x) = [x0^2, x1^2, x2^2, x0x1, x0x2, x1x2,
   x0, x1, x2, 1] (x centered) and per-gaussian coefficients c_n (with
   -0.5 and log w_n folded in), exponent[b, n] = f(x_b) . c_n — a
   TensorEngine matmul followed by exp + row-sum on the ScalarEngine
   (one activation instruction with accum_out). fp32 operands are each
   split into 3 bf16 components and the 6 significant cross products
   stacked along the contraction dim (K=60) -> ~fp32 precision.

2. Certified spatial culling: the exp work (B*N/8 elements per core on
   the only engine with an exp LUT) is the roofline. Points are
   Morton-sorted into 256 tiles of 128; for each tile, gaussians whose
   maximum possible contribution (upper bound via distance to the tile
   bbox and the largest covariance eigenvalue) is negligible are
   dropped. The drop budget is adaptive: sum of dropped upper bounds
   <= 1e-3 * (certified lower bound of the output anywhere in the
   tile), so the relative error is bounded by ~1e-3 per point by
   construction, for any input. Survivors (~12-15% here) are gathered
   on the host into dense per-tile coefficient slabs.

SPMD scheduling: all 8 cores share one instruction stream, so tile
work units (split at 2048 columns) are sorted by width and dealt
round-robin into groups of 8 — one slot per group, slot width = group
max. Per-core work is identical and balanced; each core's packed
operands carry its own unit's features/coefficients. Padding columns
encode exponent -30000 so they contribute exp(-30000) = 0.

Sharding: slots are data-parallel across 8 cores; no collectives.
"""

import sys

import numpy as np

try:
    import concourse.bass as bass  # noqa: F401
except ImportError:
    sys.path.insert(0, "/opt/trn_rl_repo")

import ml_dtypes

import concourse.bacc as bacc
import concourse.bass as bass  # noqa: F401
import concourse.mybir as mybir
import concourse.tile as tile
from concourse.bass_utils import run_bass_kernel_spmd

B, N = 32768, 4096
N_CORES = 8
TILE_PTS = 128                  # points per tile (PSUM partition dim)
N_TILES = B // TILE_PTS         # 256 spatial point-tiles
MAX_W = 2048                    # max slot width (half of PSUM, 4 banks)
KSPLIT = 60                     # 6 bf16 cross products x 10 features
CENTER = 5.0
CULL_REL = 1e-3                 # culling error budget vs per-tile lower bound
PAD_EXP = -30000.0              # exponent encoded by padding columns

BF16 = mybir.dt.bfloat16
F32 = mybir.dt.float32


# ---------------------------------------------------------------- host math

def _build_coeffs(means, cov_tril, weights):
    """[N, 10] float64 coefficients c_n so that exponent = f(x') . c_n."""
    m = means.astype(np.float64) - CENTER
    ct = cov_tril.astype(np.float64)
    w = weights.astype(np.float64)
    eps = 1e-6
    L00 = np.exp(ct[:, 0]) + eps
    L11 = np.exp(ct[:, 2]) + eps
    L22 = np.exp(ct[:, 5]) + eps
    L10, L20, L21 = ct[:, 1], ct[:, 3], ct[:, 4]
    i00 = 1.0 / L00
    i11 = 1.0 / L11
    i22 = 1.0 / L22
    i10 = -L10 / (L00 * L11)
    i21 = -L21 / (L11 * L22)
    i20 = (L10 * L21 - L20 * L11) / (L00 * L11 * L22)
    A00 = i00 * i00 + i10 * i10 + i20 * i20
    A01 = i10 * i11 + i20 * i21
    A02 = i20 * i22
    A11 = i11 * i11 + i21 * i21
    A12 = i21 * i22
    A22 = i22 * i22
    Am0 = A00 * m[:, 0] + A01 * m[:, 1] + A02 * m[:, 2]
    Am1 = A01 * m[:, 0] + A11 * m[:, 1] + A12 * m[:, 2]
    Am2 = A02 * m[:, 0] + A12 * m[:, 1] + A22 * m[:, 2]
    mAm = m[:, 0] * Am0 + m[:, 1] * Am1 + m[:, 2] * Am2
    return np.stack(
        [
            -0.5 * A00, -0.5 * A11, -0.5 * A22,
            -A01, -A02, -A12,
            Am0, Am1, Am2,
            -0.5 * mAm + np.log(w),
        ],
        axis=1,
    )


def _build_feats(x):
    """[B, 10] float64 features of centered x."""
    xc = x.astype(np.float64) - CENTER
    x0, x1, x2 = xc[:, 0], xc[:, 1], xc[:, 2]
    return np.stack(
        [x0 * x0, x1 * x1, x2 * x2, x0 * x1, x0 * x2, x1 * x2,
         x0, x1, x2, np.ones_like(x0)],
        axis=1,
    )


def _split3_bf16(a64):
    """float64 -> three bf16 components with p0+p1+p2 covering ~24 bits."""
    p0 = a64.astype(ml_dtypes.bfloat16)
    r1 = a64 - p0.astype(np.float64)
    p1 = r1.astype(ml_dtypes.bfloat16)
    r2 = r1 - p1.astype(np.float64)
    p2 = r2.astype(ml_dtypes.bfloat16)
    return p0, p1, p2


def _split_stacks(f, c):
    """[60, B] and [60, N] bf16 stacks of the 6 significant cross products."""
    F0, F1, F2 = _split3_bf16(f)
    C0, C1, C2 = _split3_bf16(c)
    pairs = [(F0, C0), (F0, C1), (F1, C0), (F0, C2), (F1, C1), (F2, C0)]
    fstack = np.ascontiguousarray(
        np.concatenate([p[0].T for p in pairs], axis=0)
    ).astype(ml_dtypes.bfloat16)
    cstack = np.ascontiguousarray(
        np.concatenate([p[1].T for p in pairs], axis=0)
    ).astype(ml_dtypes.bfloat16)
    return fstack, cstack


def _kd_order(x):
    """Recursive median split (longest axis) into tiles of TILE_PTS points:
    compact bounding boxes everywhere, which is what the culling bound
    feeds on."""
    out = []

    def rec(ids):
        if len(ids) <= TILE_PTS:
            out.append(ids)
            return
        p = x[ids]
        ax = int(np.argmax(p.max(0) - p.min(0)))
        half = len(ids) // 2
        part = np.argpartition(p[:, ax], half)
        rec(ids[part[:half]])
        rec(ids[part[half:]])

    rec(np.arange(len(x)))
    return np.concatenate(out)


def _sigma_bounds(cov_tril):
    """Per-gaussian sqrt of smallest/largest eigenvalue of L L^T."""
    ct = cov_tril.astype(np.float64)
    eps = 1e-6
    L = np.zeros((N, 3, 3))
    L[:, 0, 0] = np.exp(ct[:, 0]) + eps
    L[:, 1, 1] = np.exp(ct[:, 2]) + eps
    L[:, 2, 2] = np.exp(ct[:, 5]) + eps
    L[:, 1, 0] = ct[:, 1]
    L[:, 2, 0] = ct[:, 3]
    L[:, 2, 1] = ct[:, 4]
    ev = np.linalg.eigvalsh(L @ L.transpose(0, 2, 1))
    return np.sqrt(np.maximum(ev[:, 0], 1e-30)), np.sqrt(ev[:, -1])


def _prepare(inputs):
    """Host prep: sort, cull, pack. Returns (in_maps, slot_widths, units,
    order) where units[s][c] = (tile, survivor_cols) for slot s, core c."""
    x = inputs["x"].astype(np.float64)
    means = inputs["means"].astype(np.float64)
    cov_tril = inputs["cov_tril"]
    weights = inputs["weights"].astype(np.float64)

    order = _kd_order(x)
    xs = x[order]
    tiles = xs.reshape(N_TILES, TILE_PTS, 3)
    lo = tiles.min(axis=1)
    hi = tiles.max(axis=1)

    sig_min, sig_max = _sigma_bounds(cov_tril)
    absw = np.maximum(np.abs(weights), 1e-30)

    # per-tile certified culling
    surv = []
    for t in range(N_TILES):
        dv = np.maximum(np.maximum(lo[t][None, :] - means,
                                   means - hi[t][None, :]), 0)
        dnear2 = (dv ** 2).sum(1)
        fv = np.maximum(np.abs(means - lo[t][None, :]),
                        np.abs(means - hi[t][None, :]))
        dfar2 = (fv ** 2).sum(1)
        ub = absw * np.exp(-0.5 * dnear2 / sig_max ** 2)
        out_lb = (absw * np.exp(-0.5 * dfar2 / sig_min ** 2)).sum()
        o = np.argsort(ub)
        ndrop = int(np.searchsorted(np.cumsum(ub[o]), CULL_REL * out_lb))
        kept = np.sort(o[ndrop:])
        surv.append(kept.astype(np.int64))

    # work units (tile, survivor column slice), split at MAX_W
    raw_units = []
    for t in range(N_TILES):
        cols = surv[t]
        for off in range(0, len(cols), MAX_W):
            raw_units.append((t, cols[off:off + MAX_W]))
    raw_units.sort(key=lambda u: -len(u[1]))
    n_slots = (len(raw_units) + N_CORES - 1) // N_CORES
    units = []          # units[s][c] -> (tile, cols) or None
    slot_widths = []
    for s in range(n_slots):
        grp = raw_units[s * N_CORES:(s + 1) * N_CORES]
        w = max(len(u[1]) for u in grp)
        w = ((w + 127) // 128) * 128
        slot_widths.append(w)
        grp = grp + [None] * (N_CORES - len(grp))
        units.append(grp)

    # packed per-core operands
    feats64 = _build_feats(xs)
    coeffs64 = _build_coeffs(means, cov_tril, weights)
    fstack, cstack = _split_stacks(feats64, coeffs64)      # [60,B], [60,N]
    totc = int(np.sum(slot_widths))
    pad_col = np.zeros((64,), dtype=ml_dtypes.bfloat16)
    pad_col[9] = PAD_EXP       # block-0 constant-feature row -> exp -> 0

    in_maps = []
    for c in range(N_CORES):
        ft = np.zeros((64, n_slots * TILE_PTS), dtype=ml_dtypes.bfloat16)
        cf = np.tile(pad_col[:, None], (1, totc))
        coff = 0
        for s in range(n_slots):
            u = units[s][c]
            if u is not None:
                t, cols = u
                ft[:KSPLIT, s * TILE_PTS:(s + 1) * TILE_PTS] = \
                    fstack[:, t * TILE_PTS:(t + 1) * TILE_PTS]
                cf[:KSPLIT, coff:coff + len(cols)] = cstack[:, cols]
                cf[KSPLIT:, coff:coff + len(cols)] = 0.0
            coff += slot_widths[s]
        in_maps.append({"feats": np.ascontiguousarray(ft),
                        "coeffs": np.ascontiguousarray(cf)})
    return in_maps, slot_widths, units, order


# ------------------------------------------------------------- device kernel

def _dedup_ldweights(nc):
    """Remove redundant InstLdweights: consecutive matmuls reusing the same
    stationary operand only need the first load. Only drops loads that carry
    no semaphore waits/updates and whose weights AP matches the previous
    load, with nothing but matmuls in between on the PE stream."""
    removed = 0
    for blk in nc.m.functions[0].blocks:
        keep = []
        last_sig = None
        for ins in blk.instructions:
            if getattr(ins, "engine", None) == mybir.EngineType.PE:
                tname = type(ins).__name__
                if tname == "InstLdweights":
                    sig = repr(ins.ins[0])
                    if sig == last_sig and ins.sync_info is None:
                        removed += 1
                        continue
                    last_sig = sig
                elif tname != "InstMatmult":
                    last_sig = None
            keep.append(ins)
        if removed:
            del blk.instructions[:]
            for ins in keep:
                blk.instructions.append(ins)
    return removed


_ENGINE_SEM_PREFIX = {
    mybir.EngineType.PE: "PE_",
    mybir.EngineType.Activation: "Activation_",
}


def _strip_self_waits(nc):
    """Drop same-engine semaphore waits from multi-wait PE/ACT instructions.

    Engines execute their instruction streams in order, so a wait on the
    engine's own completion semaphore is redundant whenever the instruction
    also carries the cross-engine wait that actually orders it."""
    n = 0
    for blk in nc.m.functions[0].blocks:
        for ins in blk.instructions:
            pfx = _ENGINE_SEM_PREFIX.get(getattr(ins, "engine", None))
            si = ins.sync_info
            if pfx is None or si is None or not si.on_wait:
                continue
            waits = list(si.on_wait)
            if len(waits) < 2:
                continue
            kept = [w for w in waits if not w.ant_name.startswith(pfx)]
            if kept and len(kept) < len(waits):
                si.on_wait = kept
                n += len(waits) - len(kept)
    return n


def _strip_dead_const_memsets(nc):
    """Delete framework const-AP memsets whose tensor is never read."""
    read = set()
    for blk in nc.m.functions[0].blocks:
        for ins in blk.instructions:
            for arg in getattr(ins, "ins", []) or []:
                ref = getattr(arg, "memref", None)
                if ref:
                    read.add(ref)
    removed = 0
    for blk in nc.m.functions[0].blocks:
        keep = []
        for ins in blk.instructions:
            if (type(ins).__name__ == "InstMemset"
                    and ins.sync_info is None
                    and getattr(ins.outs[0], "memref", "").startswith("const-")
                    and ins.outs[0].memref not in read):
                removed += 1
                continue
            keep.append(ins)
        if removed:
            del blk.instructions[:]
            for ins in keep:
                blk.instructions.append(ins)
    return removed


def _trim_tail_barrier(nc):
    """Drop the second all-engine barrier round at the kernel tail.

    The TileContext epilogue runs barrier -> semaphore reset -> barrier.
    The second barrier only fences engines against code that would run
    after the reset; this kernel's end block is the last block, so there
    is nothing to fence."""
    for blk in nc.m.functions[0].blocks:
        if not getattr(blk, "name", "").endswith("_end"):
            continue
        insts = list(blk.instructions)
        idx = None
        for i, ins in enumerate(insts):
            if (type(ins).__name__ == "InstISA"
                    and ins.engine == mybir.EngineType.Pool):
                idx = i
        if idx is None or idx + 1 >= len(insts):
            return 0
        tail = insts[idx + 1:]
        if any(type(t).__name__ not in ("InstDrain", "InstEventSemaphore")
               for t in tail):
            return 0
        del blk.instructions[:]
        for ins in insts[:idx + 1]:
            blk.instructions.append(ins)
        return len(tail)
    return 0


def _build_bass(slot_widths, repeats=1):
    n_slots = len(slot_widths)
    totc = int(np.sum(slot_widths))
    nc = bacc.Bacc("TRN2", target_bir_lowering=False, debug=False,
                   num_devices=N_CORES)
    feats = nc.dram_tensor("feats", [64, n_slots * TILE_PTS], BF16,
                           kind="ExternalInput")
    coeffs = nc.dram_tensor("coeffs", [64, totc], BF16,
                            kind="ExternalInput")
    out = nc.dram_tensor("out", [128, n_slots], F32, kind="ExternalOutput")

    with tile.TileContext(nc) as tc:
        with (
            tc.tile_pool(name="const", bufs=1) as const_pool,
            tc.tile_pool(name="psum", bufs=2, space="PSUM") as psum_pool,
            tc.tile_pool(name="scratch", bufs=1) as scratch_pool,
            tc.tile_pool(name="acc", bufs=1) as acc_pool,
        ):
            FT = const_pool.tile([64, n_slots * TILE_PTS], BF16, tag="FT")
            nc.sync.dma_start(FT[:], feats.ap())
            CT = const_pool.tile([64, totc], BF16, tag="CT")
            nc.sync.dma_start(CT[:], coeffs.ap())
            accs = acc_pool.tile([128, n_slots], F32, tag="accs")
            sc = scratch_pool.tile([128, MAX_W], F32, tag="sc")

            for _r in range(repeats):
                coff = 0
                for s, w in enumerate(slot_widths):
                    lhsT = FT[:, s * TILE_PTS:(s + 1) * TILE_PTS]
                    ps = psum_pool.tile([128, MAX_W], F32, tag="ps")
                    o = 0
                    while o < w:
                        cw = min(512, w - o)
                        nc.tensor.matmul(
                            ps[:, o:o + cw],
                            lhsT,
                            CT[:, coff + o:coff + o + cw],
                            start=True,
                            stop=True,
                        )
                        o += cw
                    nc.scalar.activation(
                        sc[:, :w], ps[:, :w],
                        mybir.ActivationFunctionType.Exp,
                        accum_out=accs[:, s:s + 1],
                    )
                    coff += w
            nc.sync.dma_start(out.ap(), accs[:])
    _dedup_ldweights(nc)
    _strip_self_waits(nc)
    nc.compile()
    _trim_tail_barrier(nc)
    _strip_dead_const_memsets(nc)
    return nc


# ----------------------------------------------------------------- interface

def _assemble(results, slot_widths, units, order):
    """Per-core [128, n_slots] accumulators -> full [B] output in input
    order (summing partials of tiles split across units)."""
    tile_out = np.zeros((N_TILES, TILE_PTS), dtype=np.float64)
    for s in range(len(slot_widths)):
        for c in range(N_CORES):
            u = units[s][c]
            if u is None:
                continue
            t, _cols = u
            tile_out[t] += results[c]["out"][:, s].astype(np.float64)
    out_full = np.empty(B, dtype=np.float32)
    out_full[order] = tile_out.reshape(B).astype(np.float32)
    return out_full


def _run(inputs, trace=False):
    in_maps, slot_widths, units, order = _prepare(inputs)
    nc = _build_bass(slot_widths)
    res = run_bass_kernel_spmd(
        nc, in_maps, core_ids=list(range(N_CORES)), trace=trace
    )
    return _assemble(res.results, slot_widths, units, order), res


def kernel(x, means, cov_tril, weights):
    x = np.asarray(x)
    means = np.asarray(means)
    cov_tril = np.asarray(cov_tril)
    weights = np.asarray(weights)
    assert x.shape == (B, 3) and means.shape == (N, 3)
    assert cov_tril.shape == (N, 6) and weights.shape == (N,)
    out, _ = _run(
        {"x": x, "means": means, "cov_tril": cov_tril, "weights": weights}
    )
    return out
